# revision 1
# baseline (speedup 1.0000x reference)
# Deformable Conv2d (KS=3, stride=1, pad=1) on 8 NeuronCores, Bass/Tile.
#
# Sharding: data-parallel over batch. B=8, one batch element per core; each
# core holds the full (tiny) conv weights and its own (C,H,W) image. No
# halo / collectives needed.
#
# Per-core pipeline:
#   P0  cast-DMA x into a zero-padded bf16 SBUF image xpb with an extra
#       constant-ones channel (carries the conv bias).
#   P1  offset/modulator convs as 9 shift-matmuls (contraction over 65
#       channels = 64 image + ones*bias), PSUM -> off_sb[h, ch, w].
#   P2  per-tap channel contraction u_k[h*W+w, o] = sum_c x[c,h,w]*2*W[o,c,k]
#       (stationary = x column-tile, moving = W_k), stored to a DRAM table
#       with 1 pad row on each side of every tap's (H*W, 64) block.
#   P3  offsets -> sampling positions: clip, floor (trunc trick), bilinear
#       weights * validity masks * sigmoid(mod) -> beta[jy][jx]; int16 row
#       indices idx_jy = clamp(y0+jy,0,H-1)*W + clamp(x0,-1,W-1) + 1 (+1 is
#       the pad-row shift); idx staged to DRAM and re-loaded in the DMA's
#       (i%16, i//16) wrapped layout.
#   P4  for each w-chunk, tap k, y-neighbor jy: dma_gather 128-elem rows
#       (x-pair: table rows r and r+1 in one 512B descriptor) from the u
#       table; DVE multiplies by beta (broadcast over o via 0-stride AP);
#       identity-matmul accumulates all 36 (k,jy,slot) terms into PSUM;
#       slot0+slot1 summed on evacuation into ysb[h, w, o].
#   P5  one DMA writes ysb back as (o, h, w).
#
# Gather index i lands on partition i%128, chunk i//128; indices are ordered
# i = (w-w0)*128 + h, so partition = h. All beta/base tables live in
# [h(partition), k, w] layout.

import numpy as np
import ml_dtypes
from contextlib import ExitStack

import concourse.bass as bass
import concourse.bacc as bacc
import concourse.tile as tile
import concourse.mybir as mybir
from concourse.bass_utils import run_bass_kernel_spmd
import bass_rust

FP32 = mybir.dt.float32
BF16 = mybir.dt.bfloat16
I16 = mybir.dt.int16

H = 128
C = 64
O = 64
K = 9
KS = 3
MAX_OFF = 32.0  # max(H, W)/4 with H=128
AL = mybir.AluOpType


def _ap(base, dims, offset):
    """Clone `base` AP with explicit [stride, count] dims and element offset."""
    c = base.copy()
    c.offset = offset
    c.ap = bass_rust.VecI64Pair([list(d) for d in dims])
    return c


def build_program(W=128, chunk_w=32, debug=False):
    HW = H * W
    ROWS = HW + 2          # per-tap table rows incl. 1 pad row each side
    CW = chunk_w
    NI = CW * H            # gather indices per call
    assert W % CW == 0 and NI % 128 == 0

    nc = bacc.Bacc("TRN2", target_bir_lowering=False, debug=debug)

    xin = nc.dram_tensor("xin", [C, H, W], FP32, kind="ExternalInput")
    wconv = nc.dram_tensor("wconv", [C + 1, K, 27], BF16, kind="ExternalInput")
    wmat = nc.dram_tensor("wmat", [C, K, O], BF16, kind="ExternalInput")
    ident = nc.dram_tensor("ident", [128, 128], BF16, kind="ExternalInput")
    basey = nc.dram_tensor("basey", [H, K, W], FP32, kind="ExternalInput")
    basex = nc.dram_tensor("basex", [H, K, W], FP32, kind="ExternalInput")
    utab = nc.dram_tensor("utab", [K * ROWS * O + O], FP32, kind="Internal")
    idxd = nc.dram_tensor("idxd", [2 * K * H * W], I16, kind="Internal")
    ydram = nc.dram_tensor("y", [O, H, W], FP32, kind="ExternalOutput")

    with tile.TileContext(nc) as tc, \
            tc.tile_pool(name="persist", bufs=1) as persist:

        wconv_sb = persist.tile([C + 1, K, 27], BF16, tag="wconv_sb")
        wmat_sb = persist.tile([C, K, O], BF16, tag="wmat_sb")
        ident_sb = persist.tile([128, 128], BF16, tag="ident_sb")
        basey_sb = persist.tile([H, K, W], FP32, tag="basey_sb")
        basex_sb = persist.tile([H, K, W], FP32, tag="basex_sb")
        off_sb = persist.tile([H, 27, W], FP32, tag="off_sb")
        ysb = persist.tile([H, O, W], FP32, tag="ysb")
        beta = persist.tile([H, 2, K, W, 2], FP32, tag="beta")
        zrow = persist.tile([1, K * O], FP32, tag="zrow")

        nc.sync.dma_start(out=wconv_sb[:], in_=wconv[:])
        nc.sync.dma_start(out=wmat_sb[:], in_=wmat[:])
        nc.sync.dma_start(out=ident_sb[:], in_=ident[:])
        nc.sync.dma_start(out=basey_sb[:], in_=basey[:])
        nc.sync.dma_start(out=basex_sb[:], in_=basex[:])

        # zero the per-tap pad rows of the u table
        nc.vector.memset(zrow[:], 0.0)
        nc.sync.dma_start(
            out=_ap(utab[:], [[ROWS * O, K], [1, O]], 0),
            in_=zrow[:, 0:K * O],
        )
        nc.sync.dma_start(
            out=_ap(utab[:], [[ROWS * O, K], [1, O]], (HW + 1) * O),
            in_=zrow[:, 0:K * O],
        )
        nc.sync.dma_start(
            out=_ap(utab[:], [[O, 1], [1, O]], K * ROWS * O),
            in_=zrow[:, 0:O],
        )

        with ExitStack() as imgs:
            imgp = imgs.enter_context(tc.tile_pool(name="imgp", bufs=1))
            # P0: padded bf16 image + ones channel
            xpb = imgp.tile([C + 1, 130, W + 2], BF16, tag="xpb")
            nc.vector.memset(xpb[0:C, :, :], 0.0)
            nc.vector.memset(xpb[C:C + 1, :, :], 1.0)
            nc.gpsimd.dma_start(out=xpb[0:C, 1:129, 1:W + 1], in_=xin[:])

            # ---- P1: offset/modulator convs ----
            with tc.tile_pool(name="convp", bufs=2, space="PSUM") as convp:
                TB = 4  # w columns per psum tile
                for t0 in range(0, W, TB):
                    pc = convp.tile([128, TB, 27], FP32, tag="pc")
                    for j in range(TB):
                        t = t0 + j
                        for s in range(K):
                            dy, dx = s // KS, s % KS
                            lhsT = xpb[0:C + 1, dy:dy + 128, t + dx]
                            nc.tensor.matmul(
                                pc[:, j, :], lhsT, wconv_sb[:, s, :],
                                start=(s == 0), stop=(s == K - 1),
                            )
                    # psum (j, ch) -> off_sb (ch, w)
                    nc.vector.tensor_copy(
                        off_sb[:, :, t0:t0 + TB].rearrange("p c w -> p w c"),
                        pc[:],
                    )

            # ---- P2: u tables ----
            WH = W // 2
            with (
                tc.tile_pool(name="usb", bufs=2) as upool,
                tc.tile_pool(name="up", bufs=2, space="PSUM") as upsum,
            ):
                UB = min(8, WH)  # w columns per psum tile
                for k in range(K):
                    for half in range(2):
                        u_sb = upool.tile([128, WH, O], FP32, tag="u_sb")
                        for t0 in range(0, WH, UB):
                            pu = upsum.tile([128, UB, O], FP32, tag="pu")
                            for j in range(UB):
                                t = half * WH + t0 + j
                                lhsT = xpb[0:C, 1:129, t + 1]
                                nc.tensor.matmul(
                                    pu[:, j, :], lhsT, wmat_sb[:, k, :],
                                    start=True, stop=True,
                                )
                            nc.scalar.copy(u_sb[:, t0:t0 + UB, :], pu[:])
                        nc.sync.dma_start(
                            out=_ap(utab[:], [[W * O, 128], [1, WH * O]],
                                    (k * ROWS + 1) * O + half * WH * O),
                            in_=u_sb[:],
                        )

        # ---- P3: beta weights + gather indices ----
        with tc.tile_pool(name="scr", bufs=1) as scr:
            def S(tag, dt=FP32):
                return scr.tile([H, K, W], dt, tag=tag, name=tag)

            msig = S("msig")
            py = S("py")
            px = S("px")
            tmp = S("tmp")
            fi = S("fi", I16)
            fyf = S("fyf")
            fxf = S("fxf")
            wy = S("wy")
            wx = S("wx")
            ga = S("ga")
            gb = S("gb")
            ay0 = S("ay0")
            ay1 = S("ay1")
            ax0 = S("ax0")
            ax1 = S("ax1")
            idxi = scr.tile([H, 2, K, W], I16, tag="idxi")

            ts = nc.vector.tensor_scalar
            tt = nc.vector.tensor_tensor
            stt = nc.vector.scalar_tensor_tensor

            nc.scalar.activation(msig[:], off_sb[:, 18:27, :],
                                 mybir.ActivationFunctionType.Sigmoid)
            # clip offsets, add base grid
            offv = off_sb[:, 0:18, :].rearrange("p (a b) w -> p a b w", b=2)
            ts(py[:], offv[:, 0:9, 0, :], -MAX_OFF, MAX_OFF, AL.max, AL.min)
            ts(px[:], offv[:, 0:9, 1, :], -MAX_OFF, MAX_OFF, AL.max, AL.min)
            tt(py[:], py[:], basey_sb[:], AL.add)
            tt(px[:], px[:], basex_sb[:], AL.add)

            def floor_frac(p, ff, w_frac):
                # HW DVE float->int converts round-to-nearest:
                # rint(p+63.5)-64 == floor(p) for p >= -63 (integer-p ties
                # resolve to floor or floor-1, both bilinear-equivalent).
                ts(tmp[:], p[:], 63.5, None, AL.add)
                nc.vector.tensor_copy(fi[:], tmp[:])      # fp32 -> int16 trunc
                nc.vector.tensor_copy(ff[:], fi[:])       # back to fp32
                ts(ff[:], ff[:], -64.0, None, AL.add)
                tt(w_frac[:], p[:], ff[:], AL.subtract)

            floor_frac(py, fyf, wy)
            floor_frac(px, fxf, wx)

            def edge_weights(ff, hi0, a0, a1, w_frac):
                # a0 = (1-w)*[lo<=f<=hi], a1 = w*[lo-1<=f<=hi-1], lo=0
                ts(ga[:], ff[:], 0.0, None, AL.is_ge)
                ts(gb[:], ff[:], hi0, None, AL.is_le)
                tt(ga[:], ga[:], gb[:], AL.mult)                  # valid0
                ts(a0[:], w_frac[:], -1.0, 1.0, AL.mult, AL.add)  # 1-w
                tt(a0[:], a0[:], ga[:], AL.mult)
                ts(ga[:], ff[:], -1.0, None, AL.is_ge)
                ts(gb[:], ff[:], hi0 - 1.0, None, AL.is_le)
                tt(ga[:], ga[:], gb[:], AL.mult)                  # valid1
                tt(a1[:], w_frac[:], ga[:], AL.mult)

            edge_weights(fyf, 127.0, ay0, ay1, wy)
            edge_weights(fxf, float(W - 1), ax0, ax1, wx)

            tt(ay0[:], msig[:], ay0[:], AL.mult)   # m*(1-wy)*vy0
            tt(ay1[:], msig[:], ay1[:], AL.mult)
            tt(beta[:, 0, :, :, 0], ay0[:], ax0[:], AL.mult)
            tt(beta[:, 0, :, :, 1], ay0[:], ax1[:], AL.mult)
            tt(beta[:, 1, :, :, 0], ay1[:], ax0[:], AL.mult)
            tt(beta[:, 1, :, :, 1], ay1[:], ax1[:], AL.mult)

            # indices: ga=y0c, gb=y1c, tmp=x0c+1
            ts(ga[:], fyf[:], 0.0, 127.0, AL.max, AL.min)
            ts(gb[:], fyf[:], 1.0, 0.0, AL.add, AL.max)
            ts(gb[:], gb[:], 127.0, None, AL.min)
            ts(tmp[:], fxf[:], 1.0, 0.0, AL.add, AL.max)
            ts(tmp[:], tmp[:], float(W), None, AL.min)
            stt(wy[:], ga[:], float(W), tmp[:], AL.mult, AL.add)
            nc.vector.tensor_copy(idxi[:, 0, :, :], wy[:])
            stt(wx[:], gb[:], float(W), tmp[:], AL.mult, AL.add)
            nc.vector.tensor_copy(idxi[:, 1, :, :], wx[:])

            # stage indices to DRAM in (h, jy, k, w) order
            nc.sync.dma_start(
                out=_ap(idxd[:], [[18 * W, 128], [1, 18 * W]], 0),
                in_=idxi[:],
            )

        # ---- P4: gather + weighted combine ----
        with (
            tc.tile_pool(name="bpl", bufs=1) as bpool,
            tc.tile_pool(name="apl", bufs=2) as apool,
            tc.tile_pool(name="gp", bufs=3) as gp,
            tc.tile_pool(name="tp", bufs=3) as tp,
            tc.tile_pool(name="yp", bufs=1, space="PSUM") as yp,
        ):
            # Re-load indices into the gather's wrapped layout: index i lives
            # at [i%16, i//16] with i = w*128 + h, i.e. [h%16, w*8 + h//16].
            # DMA brings (r; j=h//16, w) with contiguous w; a strided DVE copy
            # interleaves to f = w*8 + j.
            # The gather ucode runs on 8 GPSIMD cores; each reads the idx
            # image from its own 16-partition group, so the wrapped [16,
            # n/16] image must be replicated into all 8 groups.
            bplanes = {}
            for k in range(K):
                for jy in range(2):
                    bt = bpool.tile([128, 8 * W], I16, tag=f"b{k}_{jy}",
                                    name=f"b{k}_{jy}")
                    at = apool.tile([128, 8, W], I16, tag="at", name="at")
                    for grp in range(8):
                        nc.sync.dma_start(
                            out=at[16 * grp:16 * (grp + 1), :, :],
                            in_=_ap(idxd[:],
                                    [[18 * W, 16], [16 * 18 * W, 8], [1, W]],
                                    (jy * K + k) * W),
                        )
                    slb = bt[:]
                    pstr = slb.ap[0][0]
                    nc.vector.tensor_copy(
                        _ap(slb, [[pstr, 128], [1, 8], [8, W]], slb.offset),
                        at[:],
                    )
                    bplanes[(k, jy)] = bt

            NJ = max(1, CW // 4)  # accumulation sub-blocks per chunk
            JW = CW // NJ         # w columns per sub-block
            for c0 in range(0, W, CW):
                ypsums = [yp.tile([128, JW, 2, O], FP32, tag=f"yps{j}",
                                  name=f"yps{j}") for j in range(NJ)]
                n18 = 0
                for k in range(K):
                    for jy in range(2):
                        g = gp.tile([128, CW, 128], FP32, tag="g", name="g")
                        nc.gpsimd.dma_gather(
                            g[:],
                            _ap(utab[:], [[O, ROWS], [1, 2 * O]],
                                k * ROWS * O),
                            bplanes[(k, jy)][:, c0 * 8:(c0 + CW) * 8],
                            NI, NI,
                            2 * O,
                            elem_step=O,
                            single_packet=False,
                        )
                        tv = tp.tile([128, CW, 2, O], BF16, tag="tv", name="tv")
                        bview = (beta[:, jy, k, c0:c0 + CW, :]
                                 .unsqueeze(-1).broadcast_to((H, CW, 2, O)))
                        nc.vector.tensor_tensor(
                            tv[:],
                            g[:].rearrange("p c (s o) -> p c s o", s=2),
                            bview, AL.mult,
                        )
                        for j in range(NJ):
                            nc.tensor.matmul(
                                ypsums[j][:],
                                ident_sb[:],
                                tv[:, j * JW:(j + 1) * JW, :, :],
                                start=(n18 == 0), stop=(n18 == 17),
                            )
                        n18 += 1
                for j in range(NJ):
                    dst = ysb[:, :, c0 + j * JW: c0 + (j + 1) * JW]
                    nc.scalar.copy(
                        dst,
                        ypsums[j][:, :, 0, :].rearrange("p w o -> p o w"))
                    nc.vector.tensor_tensor(
                        dst,
                        ypsums[j][:, :, 1, :].rearrange("p w o -> p o w"),
                        dst, AL.add)

        # ---- P5: output ----
        nc.sync.dma_start(
            out=_ap(ydram[:], [[W, 128], [H * W, O], [1, W]], 0),
            in_=ysb[:],
        )

    nc.compile()
    return nc


def host_inputs(xb, offset_w, offset_b, mod_w, mod_b, weight, W=128):
    """Per-core input map for one batch element xb (C,H,W)."""
    wconv = np.zeros((C + 1, K, 27), np.float32)
    for s in range(K):
        dy, dx = s // KS, s % KS
        # out(h,t,j) = sum_c w[j,c,dy,dx] * xp[c, h+dy, t+dx]
        wconv[0:C, s, 0:18] = offset_w[:, :, dy, dx].T
        wconv[0:C, s, 18:27] = mod_w[:, :, dy, dx].T
    wconv[C, 0, 0:18] = offset_b
    wconv[C, 0, 18:27] = mod_b

    wmat = np.zeros((C, K, O), np.float32)
    for k in range(K):
        ky, kx = k // KS, k % KS
        wmat[:, k, :] = 2.0 * weight[:, :, ky, kx].T  # (c,o); x2 modulator fold

    hh = np.arange(H, dtype=np.float32)
    ww = np.arange(W, dtype=np.float32)
    ky = np.repeat(np.arange(KS), KS).astype(np.float32)
    kx = np.tile(np.arange(KS), KS).astype(np.float32)
    basey = np.broadcast_to(
        ky[None, :, None] + hh[:, None, None] - 1.0, (H, K, W))
    basex = np.broadcast_to(
        kx[None, :, None] + ww[None, None, :] - 1.0, (H, K, W))

    return {
        "xin": np.ascontiguousarray(xb, np.float32),
        "wconv": wconv.astype(ml_dtypes.bfloat16),
        "wmat": wmat.astype(ml_dtypes.bfloat16),
        "ident": np.eye(128, dtype=ml_dtypes.bfloat16),
        "basey": np.ascontiguousarray(basey, np.float32),
        "basex": np.ascontiguousarray(basex, np.float32),
    }


_prog_cache = {}


def _get_program(W=128, chunk_w=32):
    key = (W, chunk_w)
    if key not in _prog_cache:
        _prog_cache[key] = build_program(W=W, chunk_w=chunk_w)
    return _prog_cache[key]


def kernel(x, offset_w, offset_b, mod_w, mod_b, weight, trace=False):
    x = np.asarray(x, np.float32)
    B = x.shape[0]
    Wd = x.shape[3]
    nc = _get_program(W=Wd, chunk_w=min(32, Wd))
    in_maps = [
        host_inputs(x[b], np.asarray(offset_w, np.float32),
                    np.asarray(offset_b, np.float32),
                    np.asarray(mod_w, np.float32),
                    np.asarray(mod_b, np.float32),
                    np.asarray(weight, np.float32), W=Wd)
        for b in range(B)
    ]
    res = run_bass_kernel_spmd(nc, in_maps, list(range(B)), trace=trace)
    y = np.stack([res.results[b]["y"] for b in range(B)]).astype(np.float32)
    if trace:
        kernel.last_result = res
    return y



# revision 7
# speedup vs baseline: 1.7092x; 1.7092x over previous
# Deformable Conv2d (KS=3, stride=1, pad=1) on 8 NeuronCores, Bass/Tile.
#
# Sharding: data-parallel over batch. B=8, one batch element per core; each
# core holds the full (tiny) conv weights and its own (C,H,W) image. No
# halo / collectives needed.
#
# V2: the bottleneck on TRN2 is GPSIMD descriptor generation for dma_gather
# (~8 ns/index), not DMA bandwidth. So each gather index now fetches ALL FOUR
# bilinear neighbors of a tap sample in one 512 B descriptor, from a
# pair-interleaved bf16 u-table:
#   utab_k unit (y, x) = [u_k(y, x, 0:64) ; u_k(y+1, x, 0:64)]   (256 B)
#   descriptor for idx = y0c*W + x0c covers units (idx, idx+1) via
#   elem_step=128 < elem_size=256  ->  slots [A=(y0,x0) B=(y0+1,x0)
#   C=(y0,x0+1) D=(y0+1,x0+1)].
# This halves gather indices vs the 2-point x-pair scheme (9 per output
# position instead of 18) and halves gather DMA bytes (bf16 table).
#
# Clamped-block betas: the fetched block is (y0c, x0c) with y0c=clamp(y0,
# 0,127), x0c=clamp(x0,0,126); when y0==-1 or x0 in {-1,127} the true
# neighbor sits in a different slot of the clamped block, so the bilinear
# slot weights are swapped with indicator corrections instead of padding
# the table.
#
# Per-core pipeline:
#   P0  cast-DMA x into a zero-padded bf16 SBUF image xpb with an extra
#       constant-ones channel (carries the conv bias).
#   P1  offset/modulator convs as 9 shift-matmuls, PSUM -> off_sb[h, ch, w].
#   P2  per-tap slot-pair tables: two matmuls per w-column (row windows
#       1:129 and 2:130 give u(y,·) and u(y+1,·); the padded row 129 makes
#       u(128,·)=0), evacuated to bf16 and written as one contiguous DMA
#       per (tap, half).
#   P3  offsets -> betas (4 slot planes, bf16) + int16 gather indices.
#   P4  per (w-chunk, tap): dma_gather 256-elem 4-slot rows; DVE multiplies
#       by beta and pre-sums the two y-slots; identity-matmul accumulates
#       the 9 taps into PSUM; x-slot pair summed on evacuation into
#       ysb[h, o, w].
#   P5  one DMA writes ysb back as (o, h, w).
#
# Gather index i lands on partition i%128, chunk i//128; indices are ordered
# i = (w-w0)*128 + h, so partition = h. All beta/base tables live in
# [h(partition), k, w] layout.

import numpy as np
import ml_dtypes
from contextlib import ExitStack

import concourse.bass as bass
import concourse.bacc as bacc
import concourse.tile as tile
import concourse.mybir as mybir
from concourse.bass_utils import run_bass_kernel_spmd
import bass_rust

FP32 = mybir.dt.float32
BF16 = mybir.dt.bfloat16
I16 = mybir.dt.int16

H = 128
C = 64
O = 64
K = 9
KS = 3
TU = 2 * O             # table unit: [u(y,x,:), u(y+1,x,:)]
MAX_OFF = 32.0         # max(H, W)/4 with H=128
AL = mybir.AluOpType


def _ap(base, dims, offset):
    """Clone `base` AP with explicit [stride, count] dims and element offset."""
    c = base.copy()
    c.offset = offset
    c.ap = bass_rust.VecI64Pair([list(d) for d in dims])
    return c


def build_program(W=128, chunk_w=32, debug=False, dump=False):
    HW = H * W
    CW = chunk_w
    NI = CW * H            # gather indices per call
    assert W % CW == 0 and NI % 128 == 0

    nc = bacc.Bacc("TRN2", target_bir_lowering=False, debug=debug)
    if dump:
        dbg_off = nc.dram_tensor("dbg_off", [H, 27, W], FP32,
                                 kind="ExternalOutput")
        dbg_beta = nc.dram_tensor("dbg_beta", [H, K, W, 4], BF16,
                                  kind="ExternalOutput")

    xin = nc.dram_tensor("xin", [C, H, W], BF16, kind="ExternalInput")
    wconv = nc.dram_tensor("wconv", [C + 1, K, 27], BF16, kind="ExternalInput")
    wmat = nc.dram_tensor("wmat", [C, K, O], BF16, kind="ExternalInput")
    ident = nc.dram_tensor("ident", [128, 128], BF16, kind="ExternalInput")
    basey = nc.dram_tensor("basey", [H, K, W], FP32, kind="ExternalInput")
    basex = nc.dram_tensor("basex", [H, K, W], FP32, kind="ExternalInput")
    # +TU slack so the gather's overlapped strided view stays in bounds
    utabs = [nc.dram_tensor(f"utab{k}", [HW * TU + TU], BF16, kind="Internal")
             for k in range(K)]
    idxd = nc.dram_tensor("idxd", [K * H * W], I16, kind="Internal")
    ydram = nc.dram_tensor("y", [O, H, W], FP32, kind="ExternalOutput")

    with tile.TileContext(nc) as tc, \
            tc.tile_pool(name="persist", bufs=1) as persist:

        wconv_sb = persist.tile([C + 1, K, 27], BF16, tag="wconv_sb")
        wmat_sb = persist.tile([C, K, O], BF16, tag="wmat_sb")
        ident_sb = persist.tile([128, 128], BF16, tag="ident_sb")
        basey_sb = persist.tile([H, K, W], FP32, tag="basey_sb")
        basex_sb = persist.tile([H, K, W], FP32, tag="basex_sb")
        off_sb = persist.tile([H, 27, W], FP32, tag="off_sb")
        ysb = persist.tile([H, O, W], FP32, tag="ysb")
        beta = persist.tile([H, K, W, 4], BF16, tag="beta")
        zrow = persist.tile([1, TU], BF16, tag="zrow")

        nc.vector.memset(zrow[:], 0.0)
        for k in range(K):
            # zero the +TU slack unit (sim finiteness; never sampled)
            nc.sync.dma_start(
                out=_ap(utabs[k][:], [[TU, 1], [1, TU]], HW * TU),
                in_=zrow[:],
            )
        nc.sync.dma_start(out=wconv_sb[:], in_=wconv[:])
        nc.sync.dma_start(out=wmat_sb[:], in_=wmat[:])
        nc.sync.dma_start(out=ident_sb[:], in_=ident[:])
        nc.sync.dma_start(out=basey_sb[:], in_=basey[:])
        nc.sync.dma_start(out=basex_sb[:], in_=basex[:])

        with ExitStack() as imgs:
            imgp = imgs.enter_context(tc.tile_pool(name="imgp", bufs=1))
            # P0: padded bf16 image + ones channel
            xpb = imgp.tile([C + 1, 130, W + 2], BF16, tag="xpb")
            nc.vector.memset(xpb[0:C, :, :], 0.0)
            nc.vector.memset(xpb[C:C + 1, :, :], 1.0)
            nc.sync.dma_start(out=xpb[0:C, 1:129, 1:W + 1], in_=xin[:])

            # ---- P1: offset/modulator convs ----
            with tc.tile_pool(name="convp", bufs=2, space="PSUM") as convp:
                TB = 4  # w columns per psum tile
                for t0 in range(0, W, TB):
                    pc = convp.tile([128, TB, 27], FP32, tag="pc")
                    for j in range(TB):
                        t = t0 + j
                        for s in range(K):
                            dy, dx = s // KS, s % KS
                            lhsT = xpb[0:C + 1, dy:dy + 128, t + dx]
                            nc.tensor.matmul(
                                pc[:, j, :], lhsT, wconv_sb[:, s, :],
                                start=(s == 0), stop=(s == K - 1),
                            )
                    # psum (j, ch) -> off_sb (ch, w)
                    nc.vector.tensor_copy(
                        off_sb[:, :, t0:t0 + TB].rearrange("p c w -> p w c"),
                        pc[:],
                    )

            # ---- P2: pair-interleaved u tables ----
            WH = W // 2
            with (
                tc.tile_pool(name="usb", bufs=2) as upool,
                tc.tile_pool(name="up", bufs=2, space="PSUM") as upsum,
            ):
                UB = min(4, WH)  # w columns per psum tile
                for k in range(K):
                    for half in range(2):
                        u_sb = upool.tile([128, WH, 2, O], BF16, tag="u_sb")
                        for t0 in range(0, WH, UB):
                            pu = upsum.tile([128, UB, 2, O], FP32, tag="pu")
                            for j in range(UB):
                                t = half * WH + t0 + j
                                # slot 0 = u(y, t), slot 1 = u(y+1, t)
                                nc.tensor.matmul(
                                    pu[:, j, 0, :], xpb[0:C, 1:129, t + 1],
                                    wmat_sb[:, k, :], start=True, stop=True,
                                )
                                nc.tensor.matmul(
                                    pu[:, j, 1, :], xpb[0:C, 2:130, t + 1],
                                    wmat_sb[:, k, :], start=True, stop=True,
                                )
                            nc.scalar.copy(u_sb[:, t0:t0 + UB, :, :], pu[:])
                        nc.sync.dma_start(
                            out=_ap(utabs[k][:], [[W * TU, 128], [1, WH * TU]],
                                    half * WH * TU),
                            in_=u_sb[:],
                        )

        # ---- P3: beta weights + gather indices ----
        with tc.tile_pool(name="scr", bufs=1) as scr:
            def S(tag, dt=FP32):
                return scr.tile([H, K, W], dt, tag=tag, name=tag)

            msig = S("msig")
            py = S("py")
            px = S("px")
            tmp = S("tmp")
            fi = S("fi", I16)
            fyf = S("fyf")
            fxf = S("fxf")
            wy = S("wy")
            wx = S("wx")
            ga = S("ga")
            gb = S("gb")
            ay0 = S("ay0")
            ay1 = S("ay1")
            ax0 = S("ax0")
            ax1 = S("ax1")
            idxi = scr.tile([H, K, W], I16, tag="idxi")

            ts = nc.vector.tensor_scalar
            tt = nc.vector.tensor_tensor
            stt = nc.vector.scalar_tensor_tensor

            nc.scalar.activation(msig[:], off_sb[:, 18:27, :],
                                 mybir.ActivationFunctionType.Sigmoid)
            # clip offsets, add base grid
            offv = off_sb[:, 0:18, :].rearrange("p (a b) w -> p a b w", b=2)
            ts(py[:], offv[:, 0:9, 0, :], -MAX_OFF, MAX_OFF, AL.max, AL.min)
            ts(px[:], offv[:, 0:9, 1, :], -MAX_OFF, MAX_OFF, AL.max, AL.min)
            tt(py[:], py[:], basey_sb[:], AL.add)
            tt(px[:], px[:], basex_sb[:], AL.add)

            def floor_frac(p, ff, w_frac):
                # HW DVE float->int converts round-to-nearest:
                # rint(p+63.5)-64 == floor(p) for p >= -63 (integer-p ties
                # resolve to floor or floor-1, both bilinear-equivalent).
                ts(tmp[:], p[:], 63.5, None, AL.add)
                nc.vector.tensor_copy(fi[:], tmp[:])      # fp32 -> int16 trunc
                nc.vector.tensor_copy(ff[:], fi[:])       # back to fp32
                ts(ff[:], ff[:], -64.0, None, AL.add)
                tt(w_frac[:], p[:], ff[:], AL.subtract)

            floor_frac(py, fyf, wy)
            floor_frac(px, fxf, wx)

            def edge_weights(ff, hi0, a0, a1, w_frac):
                # a0 = (1-w)*[lo<=f<=hi], a1 = w*[lo-1<=f<=hi-1], lo=0
                ts(ga[:], ff[:], 0.0, None, AL.is_ge)
                ts(gb[:], ff[:], hi0, None, AL.is_le)
                tt(ga[:], ga[:], gb[:], AL.mult)                  # valid0
                ts(a0[:], w_frac[:], -1.0, 1.0, AL.mult, AL.add)  # 1-w
                tt(a0[:], a0[:], ga[:], AL.mult)
                ts(ga[:], ff[:], -1.0, None, AL.is_ge)
                ts(gb[:], ff[:], hi0 - 1.0, None, AL.is_le)
                tt(ga[:], ga[:], gb[:], AL.mult)                  # valid1
                tt(a1[:], w_frac[:], ga[:], AL.mult)

            edge_weights(fyf, 127.0, ay0, ay1, wy)
            edge_weights(fxf, float(W - 1), ax0, ax1, wx)

            tt(ay0[:], msig[:], ay0[:], AL.mult)   # m*(1-wy)*vy0
            tt(ay1[:], msig[:], ay1[:], AL.mult)

            # clamped-block slot-weight corrections
            # y: block row = clamp(y0,0,127); y0==-1 -> slot A holds row 0
            ts(ga[:], fyf[:], -1.0, None, AL.is_equal)
            tt(ga[:], ga[:], ay1[:], AL.mult)          # cy*ay1
            tt(wy[:], ay0[:], ga[:], AL.add)           # bAy
            tt(wx[:], ay1[:], ga[:], AL.subtract)      # bBy
            # x: block col = clamp(x0,0,126); x0==-1 and x0==127 swaps
            ts(ga[:], fxf[:], -1.0, None, AL.is_equal)
            tt(ga[:], ga[:], ax1[:], AL.mult)          # cx1*ax1
            ts(gb[:], fxf[:], 127.0, None, AL.is_equal)
            tt(gb[:], gb[:], ax0[:], AL.mult)          # cx2*ax0
            tt(ax0[:], ax0[:], ga[:], AL.add)
            tt(ax0[:], ax0[:], gb[:], AL.subtract)     # aAx
            tt(ax1[:], ax1[:], ga[:], AL.subtract)
            tt(ax1[:], ax1[:], gb[:], AL.add)          # aCx

            # slot betas: A=(y0,x0) B=(y1,x0) C=(y0,x1) D=(y1,x1)
            tt(beta[:, :, :, 0], wy[:], ax0[:], AL.mult)
            tt(beta[:, :, :, 1], wx[:], ax0[:], AL.mult)
            tt(beta[:, :, :, 2], wy[:], ax1[:], AL.mult)
            tt(beta[:, :, :, 3], wx[:], ax1[:], AL.mult)

            # indices: idx = clamp(y0,0,127)*W + clamp(x0,0,126)
            ts(ga[:], fyf[:], 0.0, 127.0, AL.max, AL.min)
            ts(gb[:], fxf[:], 0.0, float(W - 2), AL.max, AL.min)
            stt(tmp[:], ga[:], float(W), gb[:], AL.mult, AL.add)
            nc.vector.tensor_copy(idxi[:], tmp[:])

            # stage indices to DRAM in (h, k, w) order
            nc.sync.dma_start(
                out=_ap(idxd[:], [[K * W, 128], [1, K * W]], 0),
                in_=idxi[:],
            )
            if dump:
                nc.sync.dma_start(out=dbg_off[:], in_=off_sb[:])
                nc.sync.dma_start(out=dbg_beta[:], in_=beta[:])

        # ---- P4: gather + weighted combine ----
        with (
            tc.tile_pool(name="bpl", bufs=1) as bpool,
            tc.tile_pool(name="apl", bufs=2) as apool,
            tc.tile_pool(name="gp", bufs=2) as gp,
            tc.tile_pool(name="tp", bufs=2) as tp,
            tc.tile_pool(name="t2p", bufs=2) as t2p,
            tc.tile_pool(name="yp", bufs=1, space="PSUM") as yp,
        ):
            # Re-load indices into the gather's wrapped layout: index i lives
            # at [i%16, i//16] with i = w*128 + h, i.e. [h%16, w*8 + h//16].
            # DMA brings (r; j=h//16, w) with contiguous w; a strided DVE copy
            # interleaves to f = w*8 + j.
            # The gather ucode runs on 8 GPSIMD cores; each reads the idx
            # image from its own 16-partition group, so the wrapped [16,
            # n/16] image must be replicated into all 8 groups.
            bplanes = {}
            for k in range(K):
                bt = bpool.tile([128, 8 * W], I16, tag=f"b{k}", name=f"b{k}")
                at = apool.tile([128, 8, W], I16, tag="at", name="at")
                for grp in range(8):
                    nc.sync.dma_start(
                        out=at[16 * grp:16 * (grp + 1), :, :],
                        in_=_ap(idxd[:],
                                [[K * W, 16], [16 * K * W, 8], [1, W]],
                                k * W),
                    )
                slb = bt[:]
                pstr = slb.ap[0][0]
                nc.vector.tensor_copy(
                    _ap(slb, [[pstr, 128], [1, 8], [8, W]], slb.offset),
                    at[:],
                )
                bplanes[k] = bt

            JW = 4                # w columns per psum tile
            NJ = CW // JW         # psum tiles per chunk
            assert NJ <= 8
            for c0 in range(0, W, CW):
                ypsums = [yp.tile([128, JW, 2, O], FP32, tag=f"yps{j}",
                                  name=f"yps{j}") for j in range(NJ)]
                for k in range(K):
                    g = gp.tile([128, CW, 2 * TU], BF16, tag="g", name="g")
                    nc.gpsimd.dma_gather(
                        g[:],
                        _ap(utabs[k][:], [[TU, HW], [1, 2 * TU]], 0),
                        bplanes[k][:, c0 * 8:(c0 + CW) * 8],
                        NI, NI,
                        2 * TU,
                        elem_step=TU,
                        single_packet=False,
                    )
                    # beta multiply (4 slots), then pre-sum the y-slot pairs
                    tv = tp.tile([128, CW, 2, 2, O], BF16, tag="tv", name="tv")
                    bview = (beta[:, k, c0:c0 + CW, :]
                             .unsqueeze(-1).broadcast_to((H, CW, 4, O)))
                    nc.vector.tensor_tensor(
                        tv[:].rearrange("p c x s o -> p c (x s) o"),
                        g[:].rearrange("p c (x s o) -> p c (x s) o",
                                       x=2, s=2),
                        bview, AL.mult,
                    )
                    tv2 = t2p.tile([128, CW, 2, O], BF16, tag="tv2",
                                   name="tv2")
                    nc.vector.tensor_tensor(
                        tv2[:], tv[:, :, :, 0, :], tv[:, :, :, 1, :], AL.add,
                    )
                    for j in range(NJ):
                        nc.tensor.matmul(
                            ypsums[j][:],
                            ident_sb[:],
                            tv2[:, j * JW:(j + 1) * JW, :, :],
                            start=(k == 0), stop=(k == K - 1),
                        )
                for j in range(NJ):
                    dst = ysb[:, :, c0 + j * JW: c0 + (j + 1) * JW]
                    nc.scalar.copy(
                        dst,
                        ypsums[j][:, :, 0, :].rearrange("p w o -> p o w"))
                    nc.vector.tensor_tensor(
                        dst,
                        ypsums[j][:, :, 1, :].rearrange("p w o -> p o w"),
                        dst, AL.add)

        # ---- P5: output ----
        nc.sync.dma_start(
            out=_ap(ydram[:], [[W, 128], [H * W, O], [1, W]], 0),
            in_=ysb[:],
        )

    nc.compile()
    return nc


def host_inputs(xb, offset_w, offset_b, mod_w, mod_b, weight, W=128):
    """Per-core input map for one batch element xb (C,H,W)."""
    wconv = np.zeros((C + 1, K, 27), np.float32)
    for s in range(K):
        dy, dx = s // KS, s % KS
        # out(h,t,j) = sum_c w[j,c,dy,dx] * xp[c, h+dy, t+dx]
        wconv[0:C, s, 0:18] = offset_w[:, :, dy, dx].T
        wconv[0:C, s, 18:27] = mod_w[:, :, dy, dx].T
    wconv[C, 0, 0:18] = offset_b
    wconv[C, 0, 18:27] = mod_b

    wmat = np.zeros((C, K, O), np.float32)
    for k in range(K):
        ky, kx = k // KS, k % KS
        wmat[:, k, :] = 2.0 * weight[:, :, ky, kx].T  # (c,o); x2 modulator fold

    hh = np.arange(H, dtype=np.float32)
    ww = np.arange(W, dtype=np.float32)
    ky = np.repeat(np.arange(KS), KS).astype(np.float32)
    kx = np.tile(np.arange(KS), KS).astype(np.float32)
    basey = np.broadcast_to(
        ky[None, :, None] + hh[:, None, None] - 1.0, (H, K, W))
    basex = np.broadcast_to(
        kx[None, :, None] + ww[None, None, :] - 1.0, (H, K, W))

    return {
        "xin": np.ascontiguousarray(xb).astype(ml_dtypes.bfloat16),
        "wconv": wconv.astype(ml_dtypes.bfloat16),
        "wmat": wmat.astype(ml_dtypes.bfloat16),
        "ident": np.eye(128, dtype=ml_dtypes.bfloat16),
        "basey": np.ascontiguousarray(basey, np.float32),
        "basex": np.ascontiguousarray(basex, np.float32),
    }


_prog_cache = {}


def _get_program(W=128, chunk_w=32):
    key = (W, chunk_w)
    if key not in _prog_cache:
        _prog_cache[key] = build_program(W=W, chunk_w=chunk_w)
    return _prog_cache[key]


def kernel(x, offset_w, offset_b, mod_w, mod_b, weight, trace=False):
    x = np.asarray(x, np.float32)
    B = x.shape[0]
    Wd = x.shape[3]
    nc = _get_program(W=Wd, chunk_w=min(32, Wd))
    in_maps = [
        host_inputs(x[b], np.asarray(offset_w, np.float32),
                    np.asarray(offset_b, np.float32),
                    np.asarray(mod_w, np.float32),
                    np.asarray(mod_b, np.float32),
                    np.asarray(weight, np.float32), W=Wd)
        for b in range(B)
    ]
    res = run_bass_kernel_spmd(nc, in_maps, list(range(B)), trace=trace)
    y = np.stack([res.results[b]["y"] for b in range(B)]).astype(np.float32)
    if trace:
        kernel.last_result = res
    return y


# revision 14
# speedup vs baseline: 1.8113x; 1.0598x over previous
# Deformable Conv2d (KS=3, stride=1, pad=1) on 8 NeuronCores, Bass/Tile.
#
# Sharding: data-parallel over batch. B=8, one batch element per core; each
# core holds the full (tiny) conv weights and its own (C,H,W) image. No
# halo / collectives needed.
#
# V2: the bottleneck on TRN2 is GPSIMD descriptor generation for dma_gather
# (~8 ns/index), not DMA bandwidth. So each gather index now fetches ALL FOUR
# bilinear neighbors of a tap sample in one 512 B descriptor, from a
# pair-interleaved bf16 u-table:
#   utab_k unit (y, x) = [u_k(y, x, 0:64) ; u_k(y+1, x, 0:64)]   (256 B)
#   descriptor for idx = y0c*W + x0c covers units (idx, idx+1) via
#   elem_step=128 < elem_size=256  ->  slots [A=(y0,x0) B=(y0+1,x0)
#   C=(y0,x0+1) D=(y0+1,x0+1)].
# This halves gather indices vs the 2-point x-pair scheme (9 per output
# position instead of 18) and halves gather DMA bytes (bf16 table).
#
# Clamped-block betas: the fetched block is (y0c, x0c) with y0c=clamp(y0,
# 0,127), x0c=clamp(x0,0,126); when y0==-1 or x0 in {-1,127} the true
# neighbor sits in a different slot of the clamped block, so the bilinear
# slot weights are swapped with indicator corrections instead of padding
# the table.
#
# Per-core pipeline:
#   P0  cast-DMA x into a zero-padded bf16 SBUF image xpb with an extra
#       constant-ones channel (carries the conv bias).
#   P1  offset/modulator convs as 9 shift-matmuls, PSUM -> off_sb[h, ch, w].
#   P2  per-tap slot-pair tables: two matmuls per w-column (row windows
#       1:129 and 2:130 give u(y,·) and u(y+1,·); the padded row 129 makes
#       u(128,·)=0), evacuated to bf16 and written as one contiguous DMA
#       per (tap, half).
#   P3  offsets -> betas (4 slot planes, bf16) + int16 gather indices.
#   P4  per (w-chunk, tap): dma_gather 256-elem 4-slot rows; DVE multiplies
#       by beta and pre-sums the two y-slots; identity-matmul accumulates
#       the 9 taps into PSUM; x-slot pair summed on evacuation into
#       ysb[h, o, w].
#   P5  one DMA writes ysb back as (o, h, w).
#
# Gather index i lands on partition i%128, chunk i//128; indices are ordered
# i = (w-w0)*128 + h, so partition = h. All beta/base tables live in
# [h(partition), k, w] layout.

import numpy as np
import ml_dtypes
from contextlib import ExitStack

import concourse.bass as bass
import concourse.bacc as bacc
import concourse.tile as tile
import concourse.mybir as mybir
from concourse.bass_utils import run_bass_kernel_spmd
import bass_rust

FP32 = mybir.dt.float32
BF16 = mybir.dt.bfloat16
I16 = mybir.dt.int16

H = 128
C = 64
O = 64
K = 9
KS = 3
TU = 2 * O             # table unit: [u(y,x,:), u(y+1,x,:)]
MAX_OFF = 32.0         # max(H, W)/4 with H=128
AL = mybir.AluOpType


def _ap(base, dims, offset):
    """Clone `base` AP with explicit [stride, count] dims and element offset."""
    c = base.copy()
    c.offset = offset
    c.ap = bass_rust.VecI64Pair([list(d) for d in dims])
    return c


def build_program(W=128, chunk_w=32, debug=False, dump=False):
    HW = H * W
    CW = chunk_w
    NI = CW * H            # gather indices per call
    assert W % CW == 0 and NI % 128 == 0

    nc = bacc.Bacc("TRN2", target_bir_lowering=False, debug=debug)
    if dump:
        dbg_off = nc.dram_tensor("dbg_off", [H, 27, W], FP32,
                                 kind="ExternalOutput")
        dbg_beta = nc.dram_tensor("dbg_beta", [H, K, W, 4], BF16,
                                  kind="ExternalOutput")

    xin = nc.dram_tensor("xin", [C, H, W], BF16, kind="ExternalInput")
    wconv = nc.dram_tensor("wconv", [C + 1, K, 27], BF16, kind="ExternalInput")
    wmat = nc.dram_tensor("wmat", [C, K, O], BF16, kind="ExternalInput")
    ident = nc.dram_tensor("ident", [128, 128], BF16, kind="ExternalInput")
    basey = nc.dram_tensor("basey", [H, K, W], FP32, kind="ExternalInput")
    basex = nc.dram_tensor("basex", [H, K, W], FP32, kind="ExternalInput")
    # +TU slack so the gather's overlapped strided view stays in bounds
    utabs = [nc.dram_tensor(f"utab{k}", [HW * TU + TU], BF16, kind="Internal")
             for k in range(K)]
    idxd = nc.dram_tensor("idxd", [K * H * W], I16, kind="Internal")
    ydram = nc.dram_tensor("y", [O, H, W], FP32, kind="ExternalOutput")

    with tile.TileContext(nc) as tc, \
            tc.tile_pool(name="persist", bufs=1) as persist, \
            tc.tile_pool(name="bpl", bufs=1) as bpool, \
            tc.tile_pool(name="apl", bufs=2) as apool:

        wconv_sb = persist.tile([C + 1, K, 27], BF16, tag="wconv_sb")
        wmat_sb = persist.tile([C, K, O], BF16, tag="wmat_sb")
        ident_sb = persist.tile([128, 128], BF16, tag="ident_sb")
        basey_sb = persist.tile([H, K, W], FP32, tag="basey_sb")
        basex_sb = persist.tile([H, K, W], FP32, tag="basex_sb")
        off_sb = persist.tile([H, 27, W], FP32, tag="off_sb")
        ysb = persist.tile([H, O, W], FP32, tag="ysb")
        beta = persist.tile([H, K, W, 4], BF16, tag="beta")
        zrow = persist.tile([1, TU], BF16, tag="zrow")

        nc.vector.memset(zrow[:], 0.0)
        for k in range(K):
            # zero the +TU slack unit (sim finiteness; never sampled)
            nc.sync.dma_start(
                out=_ap(utabs[k][:], [[TU, 1], [1, TU]], HW * TU),
                in_=zrow[:],
            )
        nc.sync.dma_start(out=wconv_sb[:], in_=wconv[:])
        nc.sync.dma_start(out=wmat_sb[:], in_=wmat[:])
        nc.sync.dma_start(out=ident_sb[:], in_=ident[:])
        nc.sync.dma_start(out=basey_sb[:], in_=basey[:])
        nc.sync.dma_start(out=basex_sb[:], in_=basex[:])

        with ExitStack() as imgs:
            imgp = imgs.enter_context(tc.tile_pool(name="imgp", bufs=1))
            # P0: padded bf16 image + ones channel
            xpb = imgp.tile([C + 1, 130, W + 2], BF16, tag="xpb")
            nc.vector.memset(xpb[0:C, :, :], 0.0)
            nc.vector.memset(xpb[C:C + 1, :, :], 1.0)
            nc.sync.dma_start(out=xpb[0:C, 1:129, 1:W + 1], in_=xin[:])

            # ---- P1: offset/modulator convs ----
            with tc.tile_pool(name="convp", bufs=2, space="PSUM") as convp:
                TB = 4  # w columns per psum tile
                for t0 in range(0, W, TB):
                    pc = convp.tile([128, TB, 27], FP32, tag="pc")
                    for j in range(TB):
                        t = t0 + j
                        for s in range(K):
                            dy, dx = s // KS, s % KS
                            lhsT = xpb[0:C + 1, dy:dy + 128, t + dx]
                            nc.tensor.matmul(
                                pc[:, j, :], lhsT, wconv_sb[:, s, :],
                                start=(s == 0), stop=(s == K - 1),
                            )
                    # psum (j, ch) -> off_sb (ch, w)
                    nc.vector.tensor_copy(
                        off_sb[:, :, t0:t0 + TB].rearrange("p c w -> p w c"),
                        pc[:],
                    )

            # ---- P3: beta weights + gather indices (before P2 so the
            # gather-side staging is ready while the tables build) ----
            with tc.tile_pool(name="scr", bufs=1) as scr:
                def S(tag, dt=FP32):
                    return scr.tile([H, K, W], dt, tag=tag, name=tag)

                msig = S("msig")
                py = S("py")
                px = S("px")
                tmp = S("tmp")
                fi = S("fi", I16)
                fyf = S("fyf")
                fxf = S("fxf")
                wy = S("wy")
                wx = S("wx")
                ga = S("ga")
                gb = S("gb")
                ay0 = S("ay0")
                ay1 = S("ay1")
                ax0 = S("ax0")
                ax1 = S("ax1")
                idxi = scr.tile([H, K, W], I16, tag="idxi")

                ts = nc.vector.tensor_scalar
                tt = nc.vector.tensor_tensor
                stt = nc.vector.scalar_tensor_tensor

                nc.scalar.activation(msig[:], off_sb[:, 18:27, :],
                                     mybir.ActivationFunctionType.Sigmoid)
                # clip offsets, add base grid
                offv = off_sb[:, 0:18, :].rearrange("p (a b) w -> p a b w",
                                                    b=2)
                ts(py[:], offv[:, 0:9, 0, :], -MAX_OFF, MAX_OFF, AL.max,
                   AL.min)
                ts(px[:], offv[:, 0:9, 1, :], -MAX_OFF, MAX_OFF, AL.max,
                   AL.min)
                tt(py[:], py[:], basey_sb[:], AL.add)
                tt(px[:], px[:], basex_sb[:], AL.add)

                def floor_frac(p, ff, w_frac):
                    # HW DVE float->int converts round-to-nearest:
                    # rint(p+63.5)-64 == floor(p) for p >= -63 (integer-p
                    # ties resolve to floor or floor-1, both
                    # bilinear-equivalent).
                    ts(tmp[:], p[:], 63.5, None, AL.add)
                    nc.vector.tensor_copy(fi[:], tmp[:])   # fp32 -> int16
                    nc.vector.tensor_copy(ff[:], fi[:])    # back to fp32
                    ts(ff[:], ff[:], -64.0, None, AL.add)
                    tt(w_frac[:], p[:], ff[:], AL.subtract)

                floor_frac(py, fyf, wy)
                floor_frac(px, fxf, wx)

                def edge_weights(ff, hi0, a0, a1, w_frac):
                    # a0 = (1-w)*[lo<=f<=hi], a1 = w*[lo-1<=f<=hi-1], lo=0
                    ts(ga[:], ff[:], 0.0, None, AL.is_ge)
                    ts(gb[:], ff[:], hi0, None, AL.is_le)
                    tt(ga[:], ga[:], gb[:], AL.mult)                # valid0
                    ts(a0[:], w_frac[:], -1.0, 1.0, AL.mult, AL.add)
                    tt(a0[:], a0[:], ga[:], AL.mult)
                    ts(ga[:], ff[:], -1.0, None, AL.is_ge)
                    ts(gb[:], ff[:], hi0 - 1.0, None, AL.is_le)
                    tt(ga[:], ga[:], gb[:], AL.mult)                # valid1
                    tt(a1[:], w_frac[:], ga[:], AL.mult)

                edge_weights(fyf, 127.0, ay0, ay1, wy)
                edge_weights(fxf, float(W - 1), ax0, ax1, wx)

                tt(ay0[:], msig[:], ay0[:], AL.mult)   # m*(1-wy)*vy0
                tt(ay1[:], msig[:], ay1[:], AL.mult)

                # clamped-block slot-weight corrections
                # y: block row = clamp(y0,0,127); y0==-1 -> slot A = row 0
                ts(ga[:], fyf[:], -1.0, None, AL.is_equal)
                tt(ga[:], ga[:], ay1[:], AL.mult)          # cy*ay1
                tt(wy[:], ay0[:], ga[:], AL.add)           # bAy
                tt(wx[:], ay1[:], ga[:], AL.subtract)      # bBy
                # x: block col = clamp(x0,0,126); x0 in {-1,127} swaps
                ts(ga[:], fxf[:], -1.0, None, AL.is_equal)
                tt(ga[:], ga[:], ax1[:], AL.mult)          # cx1*ax1
                ts(gb[:], fxf[:], 127.0, None, AL.is_equal)
                tt(gb[:], gb[:], ax0[:], AL.mult)          # cx2*ax0
                tt(ax0[:], ax0[:], ga[:], AL.add)
                tt(ax0[:], ax0[:], gb[:], AL.subtract)     # aAx
                tt(ax1[:], ax1[:], ga[:], AL.subtract)
                tt(ax1[:], ax1[:], gb[:], AL.add)          # aCx

                # slot betas: A=(y0,x0) B=(y1,x0) C=(y0,x1) D=(y1,x1)
                tt(beta[:, :, :, 0], wy[:], ax0[:], AL.mult)
                tt(beta[:, :, :, 1], wx[:], ax0[:], AL.mult)
                tt(beta[:, :, :, 2], wy[:], ax1[:], AL.mult)
                tt(beta[:, :, :, 3], wx[:], ax1[:], AL.mult)

                # indices: idx = clamp(y0,0,127)*W + clamp(x0,0,126)
                ts(ga[:], fyf[:], 0.0, 127.0, AL.max, AL.min)
                ts(gb[:], fxf[:], 0.0, float(W - 2), AL.max, AL.min)
                stt(tmp[:], ga[:], float(W), gb[:], AL.mult, AL.add)
                nc.vector.tensor_copy(idxi[:], tmp[:])

                # stage indices to DRAM in (h, k, w) order (Act queue)
                nc.scalar.dma_start(
                    out=_ap(idxd[:], [[K * W, 128], [1, K * W]], 0),
                    in_=idxi[:],
                )
                if dump:
                    nc.sync.dma_start(out=dbg_off[:], in_=off_sb[:])
                    nc.sync.dma_start(out=dbg_beta[:], in_=beta[:])

            # bplane staging: re-load indices into the gather's wrapped
            # layout (see P4 comment) on the Act DMA queue, off the
            # table-write queue.
            bplanes = {}
            for k in range(K):
                bt = bpool.tile([128, 8 * W], I16, tag=f"b{k}", name=f"b{k}")
                at = apool.tile([128, 8, W], I16, tag="at", name="at")
                for grp in range(8):
                    nc.scalar.dma_start(
                        out=at[16 * grp:16 * (grp + 1), :, :],
                        in_=_ap(idxd[:],
                                [[K * W, 16], [16 * K * W, 8], [1, W]],
                                k * W),
                    )
                slb = bt[:]
                pstr = slb.ap[0][0]
                nc.vector.tensor_copy(
                    _ap(slb, [[pstr, 128], [1, 8], [8, W]], slb.offset),
                    at[:],
                )
                bplanes[k] = bt

            # ---- P2: pair-interleaved u tables (tap-batched matmuls) ----
            TB2 = 8  # w columns per staging tile
            with (
                tc.tile_pool(name="usb", bufs=2) as upool,
                tc.tile_pool(name="up", bufs=2, space="PSUM") as upsum,
            ):
                for b0 in range(0, W, TB2):
                    u9 = upool.tile([128, K, TB2, 2, O], BF16, tag="u9")
                    for ti in range(TB2):
                        t = b0 + ti
                        pp = upsum.tile([128, 2, 1024], FP32, tag="pp")
                        for s in range(2):
                            # slot s = u(y+s, t); padded row 129 gives
                            # u(128,.)=0
                            nc.tensor.matmul(
                                pp[:, s, 0:512],
                                xpb[0:C, 1 + s:129 + s, t + 1],
                                wmat_sb[:, 0:8, :], start=True, stop=True,
                            )
                            nc.tensor.matmul(
                                pp[:, s, 512:576],
                                xpb[0:C, 1 + s:129 + s, t + 1],
                                wmat_sb[:, 8, :], start=True, stop=True,
                            )
                            nc.scalar.copy(
                                u9[:, :, ti, s, :],
                                pp[:, s, 0:576].rearrange(
                                    "p (k o) -> p k o", o=O),
                            )
                    for k in range(K):
                        nc.sync.dma_start(
                            out=_ap(utabs[k][:],
                                    [[W * TU, 128], [TU, TB2], [1, TU]],
                                    b0 * TU),
                            in_=u9[:, k, :, :, :],
                        )

        # ---- P4: gather + weighted combine ----
        # Gather index i lands at idx image position [i%16, i//16] with
        # i = w*128 + h, i.e. [h%16, w*8 + h//16]; the bplane staging above
        # built that layout, replicated into all 8 GPSIMD core groups.
        with (
            tc.tile_pool(name="gp", bufs=2) as gp,
            tc.tile_pool(name="tp", bufs=2) as tp,
            tc.tile_pool(name="t2p", bufs=2) as t2p,
            tc.tile_pool(name="yp", bufs=1, space="PSUM") as yp,
        ):
            JW = 4                # w columns per psum tile
            NJ = CW // JW         # psum tiles per chunk
            assert NJ <= 8
            for c0 in range(0, W, CW):
                ypsums = [yp.tile([128, JW, 2, O], FP32, tag=f"yps{j}",
                                  name=f"yps{j}") for j in range(NJ)]
                for k in range(K):
                    g = gp.tile([128, CW, 2 * TU], BF16, tag="g", name="g")
                    nc.gpsimd.dma_gather(
                        g[:],
                        _ap(utabs[k][:], [[TU, HW], [1, 2 * TU]], 0),
                        bplanes[k][:, c0 * 8:(c0 + CW) * 8],
                        NI, NI,
                        2 * TU,
                        elem_step=TU,
                        single_packet=False,
                    )
                    # beta multiply (4 slots), then pre-sum the y-slot pairs
                    tv = tp.tile([128, CW, 2, 2, O], BF16, tag="tv", name="tv")
                    bview = (beta[:, k, c0:c0 + CW, :]
                             .unsqueeze(-1).broadcast_to((H, CW, 4, O)))
                    nc.vector.tensor_tensor(
                        tv[:].rearrange("p c x s o -> p c (x s) o"),
                        g[:].rearrange("p c (x s o) -> p c (x s) o",
                                       x=2, s=2),
                        bview, AL.mult,
                    )
                    tv2 = t2p.tile([128, CW, 2, O], BF16, tag="tv2",
                                   name="tv2")
                    nc.vector.tensor_tensor(
                        tv2[:], tv[:, :, :, 0, :], tv[:, :, :, 1, :], AL.add,
                    )
                    for j in range(NJ):
                        nc.tensor.matmul(
                            ypsums[j][:],
                            ident_sb[:],
                            tv2[:, j * JW:(j + 1) * JW, :, :],
                            start=(k == 0), stop=(k == K - 1),
                        )
                for j in range(NJ):
                    dst = ysb[:, :, c0 + j * JW: c0 + (j + 1) * JW]
                    nc.scalar.copy(
                        dst,
                        ypsums[j][:, :, 0, :].rearrange("p w o -> p o w"))
                    nc.vector.tensor_tensor(
                        dst,
                        ypsums[j][:, :, 1, :].rearrange("p w o -> p o w"),
                        dst, AL.add)

        # ---- P5: output ----
        nc.sync.dma_start(
            out=_ap(ydram[:], [[W, 128], [H * W, O], [1, W]], 0),
            in_=ysb[:],
        )

    nc.compile()
    return nc


def host_inputs(xb, offset_w, offset_b, mod_w, mod_b, weight, W=128):
    """Per-core input map for one batch element xb (C,H,W)."""
    wconv = np.zeros((C + 1, K, 27), np.float32)
    for s in range(K):
        dy, dx = s // KS, s % KS
        # out(h,t,j) = sum_c w[j,c,dy,dx] * xp[c, h+dy, t+dx]
        wconv[0:C, s, 0:18] = offset_w[:, :, dy, dx].T
        wconv[0:C, s, 18:27] = mod_w[:, :, dy, dx].T
    wconv[C, 0, 0:18] = offset_b
    wconv[C, 0, 18:27] = mod_b

    wmat = np.zeros((C, K, O), np.float32)
    for k in range(K):
        ky, kx = k // KS, k % KS
        wmat[:, k, :] = 2.0 * weight[:, :, ky, kx].T  # (c,o); x2 modulator fold

    hh = np.arange(H, dtype=np.float32)
    ww = np.arange(W, dtype=np.float32)
    ky = np.repeat(np.arange(KS), KS).astype(np.float32)
    kx = np.tile(np.arange(KS), KS).astype(np.float32)
    basey = np.broadcast_to(
        ky[None, :, None] + hh[:, None, None] - 1.0, (H, K, W))
    basex = np.broadcast_to(
        kx[None, :, None] + ww[None, None, :] - 1.0, (H, K, W))

    return {
        "xin": np.ascontiguousarray(xb).astype(ml_dtypes.bfloat16),
        "wconv": wconv.astype(ml_dtypes.bfloat16),
        "wmat": wmat.astype(ml_dtypes.bfloat16),
        "ident": np.eye(128, dtype=ml_dtypes.bfloat16),
        "basey": np.ascontiguousarray(basey, np.float32),
        "basex": np.ascontiguousarray(basex, np.float32),
    }


_prog_cache = {}


def _get_program(W=128, chunk_w=32):
    key = (W, chunk_w)
    if key not in _prog_cache:
        _prog_cache[key] = build_program(W=W, chunk_w=chunk_w)
    return _prog_cache[key]


def kernel(x, offset_w, offset_b, mod_w, mod_b, weight, trace=False):
    x = np.asarray(x, np.float32)
    B = x.shape[0]
    Wd = x.shape[3]
    nc = _get_program(W=Wd, chunk_w=min(32, Wd))
    in_maps = [
        host_inputs(x[b], np.asarray(offset_w, np.float32),
                    np.asarray(offset_b, np.float32),
                    np.asarray(mod_w, np.float32),
                    np.asarray(mod_b, np.float32),
                    np.asarray(weight, np.float32), W=Wd)
        for b in range(B)
    ]
    res = run_bass_kernel_spmd(nc, in_maps, list(range(B)), trace=trace)
    y = np.stack([res.results[b]["y"] for b in range(B)]).astype(np.float32)
    if trace:
        kernel.last_result = res
    return y
